# revision 1
# baseline (speedup 1.0000x reference)
"""Multi-head attention (B=4, S=2048, D=1024, H=16) on 8 trn2 NeuronCores.

Sharding: tensor-parallel over heads — core c owns heads [2c, 2c+1]
(= model dims [128c, 128c+128)).  Each core computes q/k/v projections for
its head slice (full batch), local attention, and a partial out-projection
against its 128 columns of Wo.  The 8 partial [B*S, D] outputs are summed
on the host (the all-reduce of the Megatron pattern, done at gather time).

v2 schedule: each batch runs as two phases (qc0/qc1 = q-token halves of
1024).  A phase emits 32 "score half-units" (per k-tile and head: 2 score
matmuls + 1 exp; h0/h1 alternate so adjacent matmul pairs hit different PE
row-groups) and drains a cost-paced global filler queue between units so
the PE never idles while the ScalarE exp stream (~266us/core floor) runs:

  phase(b,qc0) fillers: v-proj(b) + v transposes, attnv(b-1,qc1) + div,
                        out-proj(b-1) tiles 8-15, first x/k/q of b+1
  phase(b,qc1) fillers: attnv(b,qc0) + div, out-proj(b) tiles 0-7,
                        remaining x loads + k/q-proj(b+1)

All PE tensors bf16 (fp8 fails the 2e-2 bar: softmax amplifies q/k noise;
exp/v quantization lands ~3% on the output).  Inputs are pre-cast to bf16
on the host so every load rides HWDGE with no SWDGE cast pass.  q/k/v/outT
double-buffer so cross-batch projections never wait on the previous
batch's readers.  Output is written bf16 (halves the store traffic); the
host accumulates the 8 partials in f32.  PSUM: scores own a 2x[128,1024]
pipe (4 banks); everything else rotates through 4x[128,512] (4 banks).
PSUM accumulation groups stay contiguous on the PE.
"""
import os
import sys

sys.path.insert(0, "/opt/trn_rl_repo")

import numpy as np

import concourse.mybir as mybir
import concourse.tile as tile
from concourse import bacc
from concourse._compat import with_exitstack
from concourse.bass_utils import run_bass_kernel_spmd
from concourse.masks import make_identity
from contextlib import ExitStack

B, S, D, H = 4, 2048, 1024, 16
HD = D // H              # 64
P = 128
NCORES = 8
NH = H // NCORES         # 2 heads per core
T = B * S                # 8192 tokens
DT = D // P              # 8 contraction tiles
KT = S // P              # 16 k-tiles per batch
QC = 1024                # q-phase width (2 psum banks)
NQC = S // QC            # 2
HQ = 512                 # matmul free-dim chunk (one psum bank)
TC = 512                 # projection token chunk
NTC = S // TC            # 4
EXP_SCALE = float(1.0 / np.sqrt(HD))

f32 = mybir.dt.float32
f32r = mybir.dt.float32r
bf16 = mybir.dt.bfloat16

LAST_EXEC_TIME_NS = None
_CACHED_NC = None


@with_exitstack
def _mha_kernel(ctx: ExitStack, tc_: tile.TileContext, ins, outs):
    nc = tc_.nc
    xT_d, wqT_d, wkT_d, wvT_d, woT_d, ones_d = ins
    out_d = outs[0]

    const = ctx.enter_context(tc_.tile_pool(name="const", bufs=1))
    wpool = ctx.enter_context(tc_.tile_pool(name="wpool", bufs=1))
    xpool = ctx.enter_context(tc_.tile_pool(name="xpool", bufs=4))
    qpool = ctx.enter_context(tc_.tile_pool(name="qpool", bufs=2))
    kpool = ctx.enter_context(tc_.tile_pool(name="kpool", bufs=2))
    vpool = ctx.enter_context(tc_.tile_pool(name="vpool", bufs=2))
    vxpool = ctx.enter_context(tc_.tile_pool(name="vxpool", bufs=2))
    epool = ctx.enter_context(tc_.tile_pool(name="epool", bufs=48))
    opool = ctx.enter_context(tc_.tile_pool(name="opool", bufs=2))
    rpool = ctx.enter_context(tc_.tile_pool(name="rpool", bufs=2))
    ospool = ctx.enter_context(tc_.tile_pool(name="ospool", bufs=3))

    # scores: dedicated 2-deep [128,1024] pipe (4 banks); misc: 4x[128,512]
    spsum = ctx.enter_context(tc_.tile_pool(name="spsum", bufs=2, space="PSUM"))
    mpsum = ctx.enter_context(tc_.tile_pool(name="mpsum", bufs=4, space="PSUM"))

    ident = const.tile([P, P], bf16, tag="ident")
    make_identity(nc, ident)

    wq = wpool.tile([P, D], bf16, tag="wq")
    wk = wpool.tile([P, D], bf16, tag="wk")
    wv = wpool.tile([P, D], bf16, tag="wv")
    wo = wpool.tile([P, D], bf16, tag="wo")

    xT_r = xT_d[:].rearrange("(dt p) t -> p dt t", p=P)

    xts = {}

    def load_x(b, t, split=False):
        if (b, t) in xts or b >= B:
            return
        if split:
            # halve the first chunk so the first projection group starts
            # after ~half the DMA instead of the whole 1MB transfer
            xt = xpool.tile([P, DT * TC], bf16, tag="xt", name=f"xt_{b}_{t}")
            xr = xt[:].rearrange("p (dt t) -> p dt t", dt=DT)
            for hf in range(2):
                nc.sync.dma_start(
                    xr[:, hf * 4:(hf + 1) * 4, :],
                    xT_r[:, hf * 4:(hf + 1) * 4,
                         b * S + t * TC: b * S + (t + 1) * TC],
                )
            xts[(b, t)] = xt
            return
        xt = xpool.tile([P, DT * TC], bf16, tag="xt", name=f"xt_{b}_{t}")
        nc.sync.dma_start(
            xt[:].rearrange("p (dt t) -> p dt t", dt=DT),
            xT_r[:, :, b * S + t * TC: b * S + (t + 1) * TC],
        )
        xts[(b, t)] = xt

    def load_w(w_sb, w_d):
        # [D, 128] DRAM -> [128, DT*128] SBUF, d-tile major
        nc.sync.dma_start(
            w_sb[:].rearrange("p (dt o) -> p dt o", dt=DT),
            w_d[:].rearrange("(dt p) o -> p dt o", p=P),
        )

    # ---------- per-batch state ----------
    st = {}

    def proj(b, w_sb, which, t):
        # one contiguous 8-matmul accumulation group + eviction
        dstT = st[(b, which)]
        pp = mpsum.tile([P, TC], f32, tag="m")
        for dt in range(DT):
            nc.tensor.matmul(
                pp[:],
                w_sb[:, dt * P:(dt + 1) * P],
                xts[(b, t)][:, dt * TC:(dt + 1) * TC],
                start=(dt == 0), stop=(dt == DT - 1),
            )
        nc.vector.tensor_copy(dstT[:, t * TC:(t + 1) * TC], pp[:])

    def vtrans2(b, kt):
        # transpose a k-tile PAIR into one shared psum tile, then land all
        # four head-blocks with a single strided copy (dest skips ones cols)
        vT = st[(b, "v")]
        v_ext = st[(b, "vx")]
        if kt == 0:
            # prefill all 32 ones-columns once per batch
            nc.vector.memset(
                v_ext[:].rearrange("p (a c) -> p a c", c=HD + 1)[:, :, HD:HD + 1],
                1.0)
        vps = mpsum.tile([P, 2 * P], bf16, tag="m")
        for j in range(2):
            nc.tensor.transpose(vps[:, j * P:(j + 1) * P],
                                vT[:, (kt + j) * P:(kt + j + 1) * P], ident[:])
        base = kt * 2 * (HD + 1)
        nc.vector.tensor_copy(
            v_ext[:, base:base + 4 * (HD + 1)]
            .rearrange("p (a c) -> p a c", c=HD + 1)[:, :, 0:HD],
            vps[:].rearrange("p (a c) -> p a c", c=HD))

    def score_half(b, qc, kt, h):
        # 2 matmuls for one head + its exp; h0/h1 half-units alternate so
        # adjacent matmuls hit different PE row-groups and fillers sit
        # between heads (covers the ACT drain lag on the 2-deep sc pipe)
        qT, kT = st[(b, "q")], st[(b, "k")]
        sc = spsum.tile([P, QC], f32, tag="sc", name=f"sc{h}_{b}_{qc}_{kt}")
        ks = slice(kt * P, (kt + 1) * P)
        hs = slice(h * HD, (h + 1) * HD)
        for half in range(QC // HQ):
            qs = slice(qc * QC + half * HQ, qc * QC + (half + 1) * HQ)
            ps_ = slice(half * HQ, (half + 1) * HQ)
            nc.tensor.matmul(sc[:, ps_], kT[hs, ks], qT[hs, qs],
                             start=True, stop=True)
        ex = epool.tile([P, QC], bf16, tag="exp", name=f"ex_{b}_{qc}_{kt}_{h}")
        nc.scalar.activation(
            ex[:], sc[:], mybir.ActivationFunctionType.Exp,
            scale=EXP_SCALE)
        st[(b, qc, "ex", h, kt)] = ex

    def attnv(b, qc, h, g, pool=None):
        # one contiguous 16-matmul accumulation group -> oe [65, 512]
        v_ext = st[(b, "vx")]
        oe = (pool or mpsum).tile([P, HQ], f32,
                                  tag="m" if pool is None else "sc",
                                  name=f"oe_{b}_{qc}_{h}_{g}")
        gs = slice(g * HQ, (g + 1) * HQ)
        for kt in range(KT):
            base = kt * 2 * (HD + 1) + h * (HD + 1)
            nc.tensor.matmul(
                oe[0:HD + 1, :],
                v_ext[:, base:base + HD + 1],
                st[(b, qc, "ex", h, kt)][:, gs],
                start=(kt == 0), stop=(kt == KT - 1),
            )
        st[(b, qc, "oe", h, g)] = oe

    def div_pre(b, qc, h, g):
        oe = st[(b, qc, "oe", h, g)]
        den = rpool.tile([1, HQ], f32, tag="den")
        # stage the denominator row to partition 0 (custom-DVE ops
        # mis-read partition-offset inputs)
        nc.vector.tensor_copy(den[:], oe[HD:HD + 1, :])
        rec = rpool.tile([1, HQ], f32, tag="rec")
        scr = rpool.tile([1, HQ], f32, tag="scr")
        nc.vector.reciprocal_approx_accurate(rec[:], den[:], scr[:])
        st[(b, qc, "recr", h, g)] = rec[:]

    def div_post(b, qc, h, g):
        oe = st.pop((b, qc, "oe", h, g))
        rec = st.pop((b, qc, "recr", h, g))
        outT = st[(b, "o")]
        # broadcast the reciprocal row on the idle GpSimd engine instead of
        # a PE matmul + PSUM->SBUF copy
        rb_sb = rpool.tile([HD, HQ], f32, tag="rbs")
        nc.gpsimd.partition_broadcast(rb_sb[:], rec, channels=HD)
        qs = slice(qc * QC + g * HQ, qc * QC + (g + 1) * HQ)
        nc.vector.tensor_mul(outT[h * HD:(h + 1) * HD, qs],
                             oe[0:HD, :], rb_sb[:])

    def oproj(b, t):
        # two t-tiles share one osb tile + one output DMA (SP-queue relief)
        outT = st[(b, "o")]
        po0 = mpsum.tile([P, HQ], f32, tag="m")
        po1 = mpsum.tile([P, HQ], f32, tag="m")
        for ec, po in ((0, po0), (1, po1)):
            nc.tensor.matmul(
                po[:],
                outT[:, t * P:(t + 1) * P],
                wo[:, ec * HQ:(ec + 1) * HQ],
                start=True, stop=True,
            )
        if t % 2 == 0:
            st[(b, "osb")] = ospool.tile([P, 2 * D], bf16, tag="osb",
                                         name=f"osb_{b}_{t}")
        osb = st[(b, "osb")]
        o2 = osb[:].rearrange("p (a d) -> p a d", a=2)
        nc.vector.tensor_copy(o2[:, t % 2, 0:HQ], po0[:])
        nc.vector.tensor_copy(o2[:, t % 2, HQ:D], po1[:])
        if t % 2 == 1:
            nc.sync.dma_start(
                out_d[b * S + (t - 1) * P: b * S + (t + 1) * P, :]
                .rearrange("(a p) d -> p a d", a=2),
                o2[:])

    def alloc_batch(b):
        if (b, "q") in st:
            return
        st[(b, "q")] = qpool.tile([P, S], bf16, tag="qT", name=f"qT{b}")
        st[(b, "k")] = kpool.tile([P, S], bf16, tag="kT", name=f"kT{b}")
        st[(b, "v")] = vpool.tile([P, S], bf16, tag="vT", name=f"vT{b}")
        st[(b, "vx")] = vxpool.tile([P, KT * 2 * (HD + 1)], bf16,
                                    tag="vext", name=f"vx{b}")
        st[(b, "o")] = opool.tile([P, S], bf16, tag="outT", name=f"oT{b}")

    fq = []   # global filler queue; leftovers spill into the next phase

    def run_phase(b, qc, fillers):
        # cost-paced: drain queue proportionally to estimated PE ns, with a
        # front-loaded lead so phase starts cover the previous phase's
        # ACT drain; at least one filler between consecutive score units
        fq.extend(fillers)
        total = sum(c for c, _ in fq) or 1
        acc = 0.0
        LEAD = 5.0
        NU = 2 * KT
        for u in range(NU):
            want = (u + 1 + LEAD) * total / (NU + LEAD)
            popped = 0
            while fq and (acc < want or popped == 0):
                c, fn = fq.pop(0)
                acc += c
                popped += 1
                fn()
            score_half(b, qc, u // 2, u % 2)

    C_PROJ, C_VT2, C_ATTNV, C_DIV, C_OPROJ, C_LDX = 1707, 110, 3413, 60, 860, 50

    def phase_fillers_qc0(b):
        f = []
        if b == 0:
            for t in range(1, NTC):
                f.append((C_PROJ, (lambda tt: lambda: proj(0, wk, "k", tt))(t)))
            f.append((C_PROJ, lambda: proj(0, wq, "q", 2)))
            f.append((C_PROJ, lambda: proj(0, wq, "q", 3)))
        # this batch: v-proj + transposes
        for t in range(NTC):
            f.append((C_PROJ, (lambda tt: lambda: proj(b, wv, "v", tt))(t)))
            for kt0 in range(t * 4, t * 4 + 4, 2):
                f.append((C_VT2, (lambda k0: lambda: vtrans2(b, k0))(kt0)))
            if t < 2 and b + 1 < B:
                alloc_batch(b + 1)
                f.append((C_LDX, (lambda tt: lambda: load_x(b + 1, tt))(t)))
        if b + 1 < B:
            f.append((C_PROJ, lambda: proj(b + 1, wk, "k", 0)))
            f.append((C_PROJ, lambda: proj(b + 1, wq, "q", 0)))
        # previous batch: attnv(qc1) + div + oproj t8-15
        if b > 0:
            for g in range(2):
                for h in range(NH):
                    f.append((C_ATTNV,
                              (lambda hh, gg: lambda: attnv(b - 1, 1, hh, gg))(h, g)))
                for h in range(NH):
                    f.append((C_DIV,
                              (lambda hh, gg: lambda: div_pre(b - 1, 1, hh, gg))(h, g)))
                for h in range(NH):
                    f.append((C_DIV,
                              (lambda hh, gg: lambda: div_post(b - 1, 1, hh, gg))(h, g)))
                for t in range(8 + g * 4, 12 + g * 4, 2):
                    f.append((C_OPROJ,
                              (lambda tt: lambda: (oproj(b - 1, tt),
                                                   oproj(b - 1, tt + 1)))(t)))
        return f

    def phase_fillers_qc1(b):
        f = []
        # this batch: attnv(qc0) + div + oproj t0-7
        for g in range(2):
            for h in range(NH):
                f.append((C_ATTNV,
                          (lambda hh, gg: lambda: attnv(b, 0, hh, gg))(h, g)))
            for h in range(NH):
                f.append((C_DIV,
                          (lambda hh, gg: lambda: div_pre(b, 0, hh, gg))(h, g)))
            for h in range(NH):
                f.append((C_DIV,
                          (lambda hh, gg: lambda: div_post(b, 0, hh, gg))(h, g)))
            for t in range(g * 4, g * 4 + 4, 2):
                f.append((C_OPROJ,
                          (lambda tt: lambda: (oproj(b, tt),
                                               oproj(b, tt + 1)))(t)))
        # next batch: remaining x loads + k/q projections
        if b + 1 < B:
            for t in range(2, NTC):
                f.append((C_LDX, (lambda tt: lambda: load_x(b + 1, tt))(t)))
            for t in range(1, NTC):
                f.append((C_PROJ, (lambda tt: lambda: proj(b + 1, wk, "k", tt))(t)))
                f.append((C_PROJ, (lambda tt: lambda: proj(b + 1, wq, "q", tt))(t)))
        return f

    # ---------- prologue ----------
    load_w(wk, wkT_d)
    load_x(0, 0)
    load_w(wq, wqT_d)
    load_x(0, 1)
    load_w(wv, wvT_d)
    load_x(0, 2)
    nc.sync.dma_start(wo[:], woT_d[:])
    load_x(0, 3)
    alloc_batch(0)
    proj(0, wk, "k", 0)
    proj(0, wq, "q", 0)
    proj(0, wq, "q", 1)

    for b in range(B):
        run_phase(b, 0, phase_fillers_qc0(b))
        run_phase(b, 1, phase_fillers_qc1(b))

    # ---------- tail: attnv(B-1, qc1) + div + oproj t8-15 ----------
    while fq:
        fq.pop(0)[1]()
    b = B - 1
    attnv(b, 1, 0, 0)
    attnv(b, 1, 1, 0)
    div_pre(b, 1, 0, 0)
    attnv(b, 1, 0, 1, pool=spsum)
    div_pre(b, 1, 1, 0)
    attnv(b, 1, 1, 1, pool=spsum)
    div_post(b, 1, 0, 0)
    div_post(b, 1, 1, 0)
    div_pre(b, 1, 0, 1)
    div_pre(b, 1, 1, 1)
    oproj(b, 8)
    oproj(b, 9)
    div_post(b, 1, 0, 1)
    div_post(b, 1, 1, 1)
    for t in (10, 11, 12, 13, 14, 15):
        oproj(b, t)


def _build():
    global _CACHED_NC
    if _CACHED_NC is not None:
        return _CACHED_NC
    nc = bacc.Bacc("TRN2", target_bir_lowering=False, debug=False)
    xT = nc.dram_tensor("xT", [D, T], bf16, kind="ExternalInput").ap()
    wqT = nc.dram_tensor("wqT", [D, P], bf16, kind="ExternalInput").ap()
    wkT = nc.dram_tensor("wkT", [D, P], bf16, kind="ExternalInput").ap()
    wvT = nc.dram_tensor("wvT", [D, P], bf16, kind="ExternalInput").ap()
    woT = nc.dram_tensor("woT", [P, D], bf16, kind="ExternalInput").ap()
    ones = nc.dram_tensor("ones", [P, HD + 2], f32, kind="ExternalInput").ap()
    out = nc.dram_tensor("out", [T, D], bf16, kind="ExternalOutput").ap()

    with tile.TileContext(nc) as tc_:
        _mha_kernel(tc_, [xT, wqT, wkT, wvT, woT, ones], [out])
    nc.compile()
    _CACHED_NC = nc
    return nc


def build_in_maps(inputs) -> list:
    import ml_dtypes
    bf = ml_dtypes.bfloat16
    x = np.asarray(inputs["x"], dtype=np.float32)
    Wq, Wk, Wv, Wo = (inputs[k] for k in ("Wq", "Wk", "Wv", "Wo"))
    xT = np.ascontiguousarray(x.reshape(T, D).T).astype(bf)   # [D, T]
    ones_in = np.ones((P, HD + 2), dtype=np.float32)

    in_maps = []
    for c in range(NCORES):
        rows = slice(c * P, (c + 1) * P)
        in_maps.append({
            "xT": xT,
            "wqT": np.ascontiguousarray(
                np.asarray(Wq, np.float32)[rows, :].T).astype(bf),
            "wkT": np.ascontiguousarray(
                np.asarray(Wk, np.float32)[rows, :].T).astype(bf),
            "wvT": np.ascontiguousarray(
                np.asarray(Wv, np.float32)[rows, :].T).astype(bf),
            "woT": np.ascontiguousarray(
                np.asarray(Wo, np.float32)[:, rows].T).astype(bf),
            "ones": ones_in,
        })
    return in_maps


def kernel(x: np.ndarray, Wq: np.ndarray, Wk: np.ndarray, Wv: np.ndarray,
           Wo: np.ndarray) -> np.ndarray:
    global LAST_EXEC_TIME_NS
    nc = _build()

    in_maps = build_in_maps(dict(x=x, Wq=Wq, Wk=Wk, Wv=Wv, Wo=Wo))

    trace = bool(os.environ.get("BASS_TRACE"))
    try:
        res = run_bass_kernel_spmd(nc, in_maps, core_ids=list(range(NCORES)),
                                   trace=trace)
    except ModuleNotFoundError:
        # no NTFF hook in this environment; run untraced
        res = run_bass_kernel_spmd(nc, in_maps, core_ids=list(range(NCORES)),
                                   trace=False)
    LAST_EXEC_TIME_NS = res.exec_time_ns

    acc = res.results[0]["out"].astype(np.float32)
    for c in range(1, NCORES):
        acc = acc + res.results[c]["out"].astype(np.float32)
    return acc.reshape(B, S, D)

